# revision 16
# baseline (speedup 1.0000x reference)
import sys
sys.path.insert(0, "/opt/trn_rl_repo")

"""v2: instruction-minimal distributed LSTM for trn2 (8 cores).

Recurrence in [b, g] layout: psum gates [64, 512] = hT-slots (lhsT, [128,64]
each) x W_hhT (rhs, resident), + gx via identity-matmul, + bias via ones-
matmul.  h computed as [64, 128], PE-transposed to the hT slice for the
next step's lhsT + XOR remote_dma_broadcast exchange.
Gate column order: (i, f, o, g) so sigmoid covers one [64, 384] span.
Phase 1 in [tok, g] layout: lhsT = xT tiles, rhs = W_ihT slice (resident).
"""
import numpy as np
import concourse.bass as bass
import concourse.mybir as mybir
from concourse import library_config, library_overlay

F32 = mybir.dt.float32
BF16 = mybir.dt.bfloat16
AF = mybir.ActivationFunctionType

B = 64
H = 1024
I = 1024
NC = 8
HS = H // NC


def build_nc(S=1024, safe_rsem=False, bf16=True, mode="full"):
    # mode: "full" (normal) | "nowait" (PE doesn't wait for remote h arrival)
    #     | "noprep" (no per-step remote DMA at all) | "phase1" (skip phase 2)
    assert mode in ("full", "nowait", "noprep", "phase1")
    ph2 = mode != "phase1"
    rsem_wait = mode == "full"
    do_exchange = mode in ("full", "nowait")
    TOK = S * B
    TOK_L = TOK // NC
    TT = TOK_L // 128          # phase-1 tiles of 128 tokens (local shard)
    assert S % 2 == 0
    nc = bass.Bass(num_devices=NC, detect_race_conditions=False,
                   num_swdge_queues=4)

    DTX = BF16 if bf16 else F32
    xT = nc.declare_dram_parameter("xT", [I, TOK_L], DTX, isOutput=False)
    wih = nc.declare_dram_parameter("wih", [128, 64 * 512], DTX, isOutput=False)
    whh = nc.declare_dram_parameter("whh", [128, 8 * 512], F32, isOutput=False)
    biasd = nc.declare_dram_parameter("bias", [1, 4096], F32, isOutput=False)
    onesd = nc.declare_dram_parameter("ones", [1, 128], F32, isOutput=False)
    identd = nc.declare_dram_parameter("ident", [64, 64], DTX, isOutput=False)
    identfd = nc.declare_dram_parameter("identf", [64, 64], F32, isOutput=False)
    zerod = nc.declare_dram_parameter("zeros", [128, 512], F32, isOutput=False)
    rankd = nc.declare_dram_parameter("rankvec", [128, 8], F32, isOutput=False)
    out = nc.declare_dram_parameter("out", [S, 64, 128], DTX, isOutput=True)
    idmap = nc.declare_dram_parameter("idmap", [128, 64], F32, isOutput=True)

    gxA = nc.dram_tensor("gxA_dram", [TOK, 512], DTX)
    gxB = nc.dram_tensor("gxB_dram", [TOK, 512], DTX)

    from contextlib import ExitStack
    es = ExitStack()
    sb = lambda n, sh, dt=F32: es.enter_context(nc.sbuf_tensor(n, sh, dt))
    ps_ = lambda n, sh: es.enter_context(nc.psum_tensor(n, sh, F32))
    sem = lambda n: es.enter_context(nc.semaphore(n))

    wih_sb = sb("wih_sb", [128, 64 * 512], DTX)
    whh_sb = sb("whh_sb", [128, 8 * 512])
    bias_sb = sb("bias_sb", [1, 4096])
    ones_sb = sb("ones_sb", [1, 128])
    ident_sb = sb("ident_sb", [64, 64], DTX)
    xtile = [sb(f"xtile{i}", [128, 8 * 128], DTX) for i in range(3)]
    stag = [sb(f"stag{i}", [128, 8 * 512], DTX) for i in range(2)]
    hbuf = [sb(f"hbuf{i}", [128, 512]) for i in range(2)]
    gxt = [sb(f"gxt{i}", [64, 512], DTX) for i in range(2)]
    gates = [sb(f"gates{i}", [64, 512]) for i in range(2)]
    tanhc = [sb(f"tanhc{i}", [64, 128]) for i in range(2)]
    hsb = [sb(f"hsb{i}", [64, 128]) for i in range(2)]
    hob = [sb(f"hob{i}", [64, 128], DTX) for i in range(2)] if bf16 else hsb
    t1 = sb("t1", [64, 128])
    t2 = sb("t2", [64, 128])
    cst = sb("cst", [64, 128])
    idbuf = sb("idbuf", [128, 64])
    ident_f32 = sb("ident_f32", [64, 64])

    p1b = [ps_(f"p1b{m}", [128, 512]) for m in range(8)]
    psg = [p1b[0], p1b[1]]                                     # rec gates [64,512]
    psh = [p1b[2], p1b[3]]                                     # hT transpose [128, :64]

    dma_w = sem("dma_w")
    dma_x = [sem(f"dma_x{i}") for i in range(3)]
    dma_gxA = [sem("dma_gxA0"), sem("dma_gxA1")]
    cc_sem = sem("cc_sem")
    dma_gx = [sem("dma_gx0"), sem("dma_gx1")]
    dma_out = sem("dma_out")
    hob_s = sem("hob_s")
    dma_id = sem("dma_id")
    pe_p1 = sem("pe_p1")
    act_p1 = sem("act_p1")
    pe_s = sem("pe_s")
    dve_s = sem("dve_s")
    act_s = sem("act_s")
    prep_s = sem("prep_s")
    rsem_all = sem("rsem_all")
    rsem = [sem(f"rsem{m}") for m in range(1, 8)] if safe_rsem else None
    lsem = [sem(f"lsem{m}") for m in range(1, 8)]
    id_rsem = [sem(f"idr{m}") for m in range(1, 8)]
    id_lsem = [sem(f"idl{m}") for m in range(1, 8)]

    INIT_DMAS = 9 * 16  # wih whh bias ones ident identf hbuf0 c rank
    LSEM_CHK = 16       # check send-drain lag every LSEM_CHK steps

    with nc.Block() as block:

        # ---------------- SYNC ----------------
        @block.sync
        def _(sync):
            sync.dma_start(out=wih_sb[:, :], in_=wih[:, :]).then_inc(dma_w, 16)
            sync.dma_start(out=whh_sb[:, :], in_=whh[:, :]).then_inc(dma_w, 16)
            sync.dma_start(out=bias_sb[:, :], in_=biasd[:, :]).then_inc(dma_w, 16)
            sync.dma_start(out=ones_sb[:, :], in_=onesd[:, :]).then_inc(dma_w, 16)
            sync.dma_start(out=ident_sb[:, :], in_=identd[:, :]).then_inc(dma_w, 16)
            sync.dma_start(out=ident_f32[:, :], in_=identfd[:, :]).then_inc(dma_w, 16)
            sync.dma_start(out=hbuf[0][:, :], in_=zerod[:, :]).then_inc(dma_w, 16)
            sync.dma_start(out=cst[:, :], in_=zerod[:64, 0:128]).then_inc(dma_w, 16)
            sync.dma_start(out=idbuf[:, 0:8], in_=rankd[:, :]).then_inc(dma_w, 16)
            # phase 1 x tiles (local shard)
            for T in range(min(3, TT)):
                sync.dma_start(
                    out=bass.AP(xtile[T % 3], 0, [[1024, 128], [128, 8], [1, 128]]),
                    in_=bass.AP(xT, T * 128, [[TOK_L, 128], [128 * TOK_L, 8], [1, 128]]),
                ).then_inc(dma_x[T % 3], 16)
            for T in range(TT):
                if T + 3 < TT:
                    sync.wait_ge(pe_p1, 8 * (T + 1))
                    sync.dma_start(
                        out=bass.AP(xtile[(T + 3) % 3], 0, [[1024, 128], [128, 8], [1, 128]]),
                        in_=bass.AP(xT, (T + 3) * 128, [[TOK_L, 128], [128 * TOK_L, 8], [1, 128]]),
                    ).then_inc(dma_x[(T + 3) % 3], 16)
                # one strided DMA: stag (8 gate-chunks of this tile) -> gxA
                # chunk j rows: j*TOK_L + T*128 .. +128
                sync.wait_ge(act_p1, 8 * (T + 1))
                sync.dma_start(
                    out=bass.AP(gxA, T * 128 * 512, [[512, 128], [TOK_L * 512, 8], [1, 512]]),
                    in_=bass.AP(stag[T % 2], 0, [[4096, 128], [512, 8], [1, 512]]),
                ).then_inc(dma_gxA[T % 2], 16)
            # phase 2 gxt prefetch: gated on the AllToAll. The out-DMA (shifted
            # 2 steps back) also lives here so it stays off the ACT engine.
            sync.wait_ge(cc_sem, 1)
            if ph2:
                for s in range(S):
                    if s >= 2:
                        sync.wait_ge(pe_s, 2 * (s - 2) + 1)
                    sync.dma_start(out=gxt[s % 2][:, :], in_=gxB[s * 64:(s + 1) * 64, :]
                                   ).then_inc(dma_gx[s % 2], 16)
                    if bf16 and s >= 2:
                        sync.wait_ge(hob_s, s - 1)
                        sync.dma_start(out=out[s - 2, :, :], in_=hob[(s - 2) % 2][:, :]
                                       ).then_inc(dma_out, 16)
                if bf16:
                    for s in (S - 2, S - 1):
                        sync.wait_ge(hob_s, s + 1)
                        sync.dma_start(out=out[s, :, :], in_=hob[s % 2][:, :]
                                       ).then_inc(dma_out, 16)

        # ---------------- PE ----------------
        @block.tensor
        def _(tensor):
            tensor.wait_ge(dma_w, INIT_DMAS)
            # phase 1: bank g = my tokens x gate-chunk g (all cores' gates)
            for T in range(TT):
                tensor.wait_ge(dma_x[T % 3], 16 * (T // 3 + 1))
                for g in range(8):
                    if T >= 1:
                        tensor.wait_ge(act_p1, 8 * (T - 1) + g + 1)
                    for j in range(8):
                        tensor.matmul(
                            p1b[g][:, :],
                            xtile[T % 3][:, j * 128:(j + 1) * 128],
                            wih_sb[:, (j * 8 + g) * 512:(j * 8 + g + 1) * 512],
                            start=(j == 0), stop=False,
                        )
                    mm = tensor.matmul(p1b[g][:, :], ones_sb[:, :],
                                       bias_sb[:, g * 512:(g + 1) * 512],
                                       start=False, stop=True)
                    mm.then_inc(pe_p1, 1)
            # phase 2
            tensor.wait_ge(act_p1, 8 * TT)
            for s in range(S if ph2 else 0):
                par = s % 2
                # gates psum [64, 512]
                tensor.wait_ge(dma_gx[par], 16 * (s // 2 + 1))
                if s >= 2:
                    tensor.wait_ge(act_s, 3 * (s - 2) + 2)   # psum WAR (acts of s-2 done)
                tensor.matmul(psg[par][0:64, :], ident_sb[:, :], gxt[par][:, :],
                              start=True, stop=False)
                if s >= 1:
                    tensor.wait_ge(dve_s, 5 * s)             # own hT slot ready
                    if rsem_wait:
                        if safe_rsem:
                            for m in range(7):
                                tensor.wait_ge(rsem[m], 2 * s)
                        else:
                            tensor.wait_ge(rsem_all, 14 * s)
                for j in range(8):
                    mm = tensor.matmul(
                        psg[par][0:64, :],
                        hbuf[par][:, j * 64:(j + 1) * 64],
                        whh_sb[:, j * 512:(j + 1) * 512],
                        start=False, stop=(j == 7),
                    )
                    if j == 7:
                        mm.then_inc(pe_s, 1)
                # wait h of this step then transpose to hT slice
                tensor.wait_ge(dve_s, 5 * s + 4)
                tensor.transpose(psh[(s + 1) % 2][:, 0:64], hsb[par][:, :], ident_f32[:, :]
                                 ).then_inc(pe_s, 1)
            if ph2:
                tensor.wait_ge(act_s, 3 * S)

        # ---------------- ACT ----------------
        @block.scalar
        def _(scalar):
            scalar.wait_ge(dma_w, INIT_DMAS)
            # phase 1 copies psum bank g -> staging chunk g (casts to DTX)
            for T in range(TT):
                if T >= 2:
                    scalar.wait_ge(dma_gxA[T % 2], 16 * (T // 2))
                for g in range(8):
                    scalar.wait_ge(pe_p1, 8 * T + g + 1)
                    scalar.activation(stag[T % 2][:, g * 512:(g + 1) * 512],
                                      p1b[g][:, :], AF.Identity).then_inc(act_p1, 1)
            # idmap
            for m in range(7):
                scalar.wait_ge(id_rsem[m], 2)
            scalar.dma_start(out=idmap[:, :], in_=idbuf[:, :]).then_inc(dma_id, 16)
            # phase 2: sigmoid [64, 384] (i,f,o), tanh g, tanh c; out DMA
            for s in range(S if ph2 else 0):
                par = s % 2
                scalar.wait_ge(pe_s, 2 * s + 1)
                scalar.activation(gates[par][:, 0:384], psg[par][0:64, 0:384], AF.Sigmoid
                                  ).then_inc(act_s, 1)
                scalar.activation(gates[par][:, 384:512], psg[par][0:64, 384:512], AF.Tanh
                                  ).then_inc(act_s, 1)
                scalar.wait_ge(dve_s, 5 * s + 3)
                scalar.activation(tanhc[par][:, :], cst[:, :], AF.Tanh).then_inc(act_s, 1)
                scalar.wait_ge(dve_s, 5 * s + 4)
                if bf16:
                    if s >= 2:
                        scalar.wait_ge(dma_out, 16 * (s - 1))
                    scalar.activation(hob[par][:, :], hsb[par][:, :], AF.Copy
                                      ).then_inc(hob_s, 1)
                else:
                    scalar.dma_start(out=out[s, :, :], in_=hob[par][:, :]
                                     ).then_inc(dma_out, 16)
            if ph2:
                scalar.wait_ge(dma_out, 16 * S)

        # ---------------- DVE ----------------
        @block.vector
        def _(vector):
            vector.wait_ge(dma_w, INIT_DMAS)
            for s in range(S if ph2 else 0):
                par = s % 2
                vector.wait_ge(act_s, 3 * s + 1)
                if s >= 1:
                    vector.wait_ge(dve_s, 5 * (s - 1) + 3)   # prev c written
                vector.tensor_mul(t1[:, :], gates[par][:, 128:256], cst[:, :]).then_inc(dve_s, 1)
                vector.wait_ge(act_s, 3 * s + 2)
                vector.tensor_mul(t2[:, :], gates[par][:, 0:128], gates[par][:, 384:512]
                                  ).then_inc(dve_s, 1)
                vector.wait_ge(dve_s, 5 * s + 2)
                vector.tensor_add(cst[:, :], t1[:, :], t2[:, :]).then_inc(dve_s, 1)
                vector.wait_ge(act_s, 3 * s + 3)
                if s >= 2:
                    if bf16:
                        vector.wait_ge(hob_s, s - 1)   # hob copy of s-2 read hsb
                    else:
                        vector.wait_ge(dma_out, 16 * (s - 1))
                vector.tensor_mul(hsb[par][:, :], gates[par][:, 256:384], tanhc[par][:, :]
                                  ).then_inc(dve_s, 1)
                # copy hT from transpose psum into hbuf slot 0 of next parity
                vector.wait_ge(pe_s, 2 * (s + 1))
                vector.tensor_copy(hbuf[(s + 1) % 2][:, 0:64], psh[(s + 1) % 2][:, 0:64]
                                   ).then_inc(dve_s, 1)

        # ---------------- GPSIMD ----------------
        @block.gpsimd
        def _(gp):
            gp.load_library(library_config.remote_dma)

            # spread the 7 bcasts across the 4 SWDGE queues so descriptor
            # generation runs on all 4 Q7 pairs in parallel
            QN = lambda m: (m - 1) % 4
            QTRIG = [2, 2, 2, 1]       # preps per queue per step

            def bcast(m, out_ap, in_ap, rs, ls):
                rdests = [None] * 8
                rdests[m] = (0, m)
                gp.remote_dma_broadcast(out_ap=out_ap, in_ap=in_ap,
                                        remote_sem=rs, local_sem=ls, rdests=rdests,
                                        queue_num=QN(m)).then_inc(prep_s, 1)

            def fire():
                for q in range(4):
                    gp.trigger_dma(count=QTRIG[q], queue_num=q)

            for m in range(1, 8):
                bcast(m, idbuf[:, m * 8:(m + 1) * 8], idbuf[:, 0:8],
                      id_rsem[m - 1], id_lsem[m - 1])
            gp.wait_ge(prep_s, 7)
            gp.wait_ge(dma_w, INIT_DMAS)
            fire()
            # redistribute gx: [my tokens, all gates] -> [all tokens, my gates]
            gp.wait_ge(dma_gxA[0], 16 * ((TT + 1) // 2))
            if TT >= 2:
                gp.wait_ge(dma_gxA[1], 16 * (TT // 2))
            gp.collective_compute(
                "AllToAll",
                mybir.AluOpType.bypass,
                replica_groups=[list(range(NC))],
                ins=[gxA.ap().opt()],
                outs=[gxB.ap().opt()],
            ).then_inc(cc_sem, 1)
            gp.wait_ge(cc_sem, 1)
            if do_exchange and ph2:
                if S >= 2:
                    for m in range(1, 8):
                        bcast(m, hbuf[1][:, m * 64:(m + 1) * 64], hbuf[1][:, 0:64],
                              rsem[m - 1] if safe_rsem else rsem_all, lsem[m - 1])
                for s in range(S):
                    gp.wait_ge(dve_s, 5 * (s + 1))
                    if s < S - 1:
                        gp.wait_ge(prep_s, 7 * (s + 2))
                        fire()
                    if s % LSEM_CHK == 0 and s >= LSEM_CHK:
                        for m in range(7):
                            gp.wait_ge(lsem[m], 16 * (s - LSEM_CHK + 1))
                    if s + 1 <= S - 2:
                        p2 = (s + 2) % 2
                        for m in range(1, 8):
                            bcast(m, hbuf[p2][:, m * 64:(m + 1) * 64], hbuf[p2][:, 0:64],
                                  rsem[m - 1] if safe_rsem else rsem_all, lsem[m - 1])
                for m in range(7):
                    gp.wait_ge(lsem[m], 16 * (S - 1))

    es.close()
    library_overlay.lower_extended_insts(nc)
    return nc


def host_prepare(x, W_ih, W_hh, b_ih, b_hh, S, umap, bf16=True):
    TOK = S * B
    TOK_L = TOK // NC
    import ml_dtypes
    dtx = ml_dtypes.bfloat16 if bf16 else np.float32
    xs = np.ascontiguousarray(x[:, :S, :])
    xT = np.ascontiguousarray(xs.transpose(2, 1, 0).reshape(I, TOK)).astype(dtx)
    # gate col order (i, f, o, g): source gate blocks in W_* rows: i=0,f=1,g=2,o=3
    gorder = [0, 1, 3, 2]
    rows_of = [np.concatenate(
        [np.arange(g * H + c * HS, g * H + (c + 1) * HS) for g in gorder])
        for c in range(NC)]
    # rank-ordered full-width W_ihT / bias: chunk c = gates of core c (same
    # for every core; the AllToAll routes by rank)
    wih_full = np.concatenate([W_ih[rows_of[c], :].T for c in range(NC)], axis=1)
    wih_packed = np.ascontiguousarray(
        wih_full.reshape(8, 128, 8, 512).transpose(1, 0, 2, 3).reshape(128, 32768)
    ).astype(dtx)
    bias_full = (b_ih + b_hh).astype(np.float32)
    bias_perm = np.concatenate([bias_full[rows_of[c]] for c in range(NC)]).reshape(1, 4096)
    in_maps = []
    for r in range(NC):
        rows = rows_of[r]
        whh_slice = W_hh[rows, :]                            # [512 g, 1024 k]
        whh_perm = np.concatenate(
            [whh_slice[:, umap[r][j] * HS:(umap[r][j] + 1) * HS].T for j in range(8)],
            axis=0)                                          # [1024 (slot-k), 512]
        in_maps.append(dict(
            xT=np.ascontiguousarray(xT[:, r * TOK_L:(r + 1) * TOK_L]),
            wih=wih_packed,
            whh=np.ascontiguousarray(whh_perm.reshape(8, 128, 512).transpose(1, 0, 2)
                                     .reshape(128, 4096)).astype(np.float32),
            bias=bias_perm,
            ones=np.ones((1, 128), np.float32),
            ident=np.eye(64).astype(dtx),
            identf=np.eye(64, dtype=np.float32),
            zeros=np.zeros((128, 512), np.float32),
            rankvec=np.full((128, 8), float(r), np.float32),
        ))
    return in_maps


def host_gather(results, S):
    outs = np.stack([np.asarray(results[r]["out"]).astype(np.float32).reshape(S, 64, 128)
                     for r in range(NC)])
    return np.ascontiguousarray(outs.transpose(2, 1, 0, 3).reshape(B, S, H))


def read_umap(results):
    return [[int(round(float(np.asarray(results[r]["idmap"]).reshape(128, 64)[0, m * 8])))
             for m in range(8)] for r in range(NC)]


DEFAULT_UMAP = [[r ^ (m if m < 4 else m ^ 2) for m in range(8)] for r in range(NC)]
IDENTITY_UMAP = [[r ^ m for m in range(8)] for r in range(NC)]



# ---------------- PJRT runner (no donation, outputs as results) ----------------

import jax
from jax.sharding import Mesh, PartitionSpec
from jax.experimental.shard_map import shard_map
from concourse import bass2jax

import jax
from jax.sharding import Mesh, PartitionSpec
from jax.experimental.shard_map import shard_map
from concourse import bass2jax


class Runner:
    def __init__(self, nc, n_cores=8, pass_out_zeros=False):
        # pass_out_zeros: ship zero buffers for ExternalOutputs (needed only
        # if the kernel does not write every output element)
        bass2jax.install_neuronx_cc_hook()
        self.nc = nc
        self.n_cores = n_cores
        partition_name = nc.partition_id_tensor.name if nc.partition_id_tensor else None
        in_names, out_names, out_avals = [], [], []
        for alloc in nc.m.functions[0].allocations:
            if not isinstance(alloc, mybir.MemoryLocationSet):
                continue
            name = alloc.memorylocations[0].name
            if alloc.kind == "ExternalInput":
                if name != partition_name:
                    in_names.append(name)
            elif alloc.kind == "ExternalOutput":
                out_names.append(name)
                out_avals.append(jax.core.ShapedArray(
                    tuple(alloc.tensor_shape), mybir.dt.np(alloc.dtype)))
        self.in_names, self.out_names, self.out_avals = in_names, out_names, out_avals
        self.pass_out_zeros = pass_out_zeros
        n_params = len(in_names)
        n_outs = len(out_names) if pass_out_zeros else 0
        in_names_all = (in_names + (out_names if pass_out_zeros else [])
                        + ([partition_name] if partition_name else []))

        def _body(*args):
            operands = list(args)
            if partition_name is not None:
                operands.append(bass2jax.partition_id_tensor())
            outs = bass2jax._bass_exec_p.bind(
                *operands, out_avals=tuple(out_avals), in_names=tuple(in_names_all),
                out_names=tuple(out_names), lowering_input_output_aliases=(),
                sim_require_finite=False, sim_require_nnan=False, nc=nc)
            return tuple(outs)

        devices = jax.devices()[:n_cores]
        self.mesh = Mesh(np.asarray(devices), ("core",))
        self.jitted = jax.jit(shard_map(
            _body, mesh=self.mesh,
            in_specs=(PartitionSpec("core"),) * (n_params + n_outs),
            out_specs=(PartitionSpec("core"),) * len(out_names), check_rep=False),
            keep_unused=True)
        self.dev_args = None

    def set_inputs(self, in_maps, only=None):
        n = self.n_cores
        if self.dev_args is None:
            n_extra = len(self.out_names) if self.pass_out_zeros else 0
            self.dev_args = [None] * (len(self.in_names) + n_extra)
            only = None
        for i, name in enumerate(self.in_names):
            if only is not None and name not in only:
                continue
            cat = np.concatenate([np.asarray(in_maps[c][name]) for c in range(n)], axis=0)
            self.dev_args[i] = jax.device_put(cat)
        if only is None and self.pass_out_zeros:
            for j, av in enumerate(self.out_avals):
                z = np.zeros((n * av.shape[0], *av.shape[1:]), av.dtype)
                self.dev_args[len(self.in_names) + j] = jax.device_put(z)

    def run(self):
        outs = self.jitted(*self.dev_args)
        jax.block_until_ready(outs)
        return outs

    def results(self, outs):
        n = self.n_cores
        res = []
        for c in range(n):
            d = {}
            for i, name in enumerate(self.out_names):
                a = np.asarray(outs[i])
                d[name] = a.reshape(n, *self.out_avals[i].shape)[c]
            res.append(d)
        return res



# ---------------- harness entry point ----------------


USE_BF16 = True      # bf16 x / W_ih / gx / output staging; recurrence fp32
_CACHE = {}


def kernel(x, W_ih, W_hh, b_ih, b_hh):
    """Full-input distributed LSTM on 8 trn2 NeuronCores. Returns (B, S, H) f32."""
    x = np.ascontiguousarray(np.asarray(x, np.float32))
    W_ih = np.asarray(W_ih, np.float32)
    W_hh = np.asarray(W_hh, np.float32)
    b_ih = np.asarray(b_ih, np.float32)
    b_hh = np.asarray(b_hh, np.float32)
    S = x.shape[1]

    if "runner" not in _CACHE:
        nc = build_nc(S, bf16=USE_BF16)
        _CACHE["runner"] = Runner(nc, NC)
        _CACHE["S"] = S
    assert _CACHE["S"] == S
    r = _CACHE["runner"]

    umap = _CACHE.get("umap", DEFAULT_UMAP)
    in_maps = host_prepare(x, W_ih, W_hh, b_ih, b_hh, S, umap, bf16=USE_BF16)
    r.set_inputs(in_maps)
    res = r.results(r.run())
    obs = read_umap(res)
    if obs != umap:
        # physical core mapping differs from assumption: rebuild the permuted
        # recurrent weights and re-execute (same compiled NEFF)
        assert all(sorted(row) == list(range(8)) for row in obs), obs
        _CACHE["umap"] = obs
        in_maps = host_prepare(x, W_ih, W_hh, b_ih, b_hh, S, obs, bf16=USE_BF16)
        r.set_inputs(in_maps, only={"whh"})
        res = r.results(r.run())
        assert read_umap(res) == obs
    return host_gather(res, S)



# revision 20
# speedup vs baseline: 16.0343x; 16.0343x over previous
import sys
sys.path.insert(0, "/opt/trn_rl_repo")

"""v2: instruction-minimal distributed LSTM for trn2 (8 cores).

Recurrence in [b, g] layout: psum gates [64, 512] = hT-slots (lhsT, [128,64]
each) x W_hhT (rhs, resident), + gx via identity-matmul, + bias via ones-
matmul.  h computed as [64, 128], PE-transposed to the hT slice for the
next step's lhsT + XOR remote_dma_broadcast exchange.
Gate column order: (i, f, o, g) so sigmoid covers one [64, 384] span.
Phase 1 in [tok, g] layout: lhsT = xT tiles, rhs = W_ihT slice (resident).
"""
import numpy as np
import concourse.bass as bass
import concourse.mybir as mybir
from concourse import library_config, library_overlay

F32 = mybir.dt.float32
BF16 = mybir.dt.bfloat16
AF = mybir.ActivationFunctionType

B = 64
H = 1024
I = 1024
NC = 8
HS = H // NC


def build_nc(S=1024, safe_rsem=False, bf16=True, mode="full", nq=1, nbcast=7,
             sem_only=False):
    # mode: "full" (normal) | "nowait" (PE doesn't wait for remote h arrival)
    #     | "noprep" (no per-step remote DMA at all) | "phase1" (skip phase 2)
    # nbcast/sem_only: timing probes (only meaningful with mode="nowait")
    assert mode in ("full", "nowait", "noprep", "phase1")
    ph2 = mode != "phase1"
    rsem_wait = mode == "full"
    do_exchange = mode in ("full", "nowait")
    assert mode == "full" or not rsem_wait
    TOK = S * B
    TOK_L = TOK // NC
    TT = TOK_L // 128          # phase-1 tiles of 128 tokens (local shard)
    assert S % 2 == 0
    nc = bass.Bass(num_devices=NC, detect_race_conditions=False,
                   num_swdge_queues=max(nq, 1))

    DTX = BF16 if bf16 else F32
    xT = nc.declare_dram_parameter("xT", [I, TOK_L], DTX, isOutput=False)
    wih = nc.declare_dram_parameter("wih", [128, 64 * 512], DTX, isOutput=False)
    whh = nc.declare_dram_parameter("whh", [128, 8 * 512], F32, isOutput=False)
    biasd = nc.declare_dram_parameter("bias", [1, 4096], F32, isOutput=False)
    onesd = nc.declare_dram_parameter("ones", [1, 128], F32, isOutput=False)
    identd = nc.declare_dram_parameter("ident", [64, 64], DTX, isOutput=False)
    identfd = nc.declare_dram_parameter("identf", [64, 64], F32, isOutput=False)
    zerod = nc.declare_dram_parameter("zeros", [128, 512], F32, isOutput=False)
    rankd = nc.declare_dram_parameter("rankvec", [128, 8], F32, isOutput=False)
    out = nc.declare_dram_parameter("out", [S, 64, 128], DTX, isOutput=True)
    idmap = nc.declare_dram_parameter("idmap", [128, 64], F32, isOutput=True)

    gxA = nc.dram_tensor("gxA_dram", [TOK, 512], DTX)
    gxB = nc.dram_tensor("gxB_dram", [TOK, 512], DTX)

    from contextlib import ExitStack
    es = ExitStack()
    sb = lambda n, sh, dt=F32: es.enter_context(nc.sbuf_tensor(n, sh, dt))
    ps_ = lambda n, sh: es.enter_context(nc.psum_tensor(n, sh, F32))
    sem = lambda n: es.enter_context(nc.semaphore(n))

    wih_sb = sb("wih_sb", [128, 64 * 512], DTX)
    whh_sb = sb("whh_sb", [128, 8 * 512])
    bias_sb = sb("bias_sb", [1, 4096])
    ones_sb = sb("ones_sb", [1, 128])
    ident_sb = sb("ident_sb", [64, 64], DTX)
    xtile = [sb(f"xtile{i}", [128, 8 * 128], DTX) for i in range(3)]
    stag = [sb(f"stag{i}", [128, 8 * 512], DTX) for i in range(2)]
    hbuf = [sb(f"hbuf{i}", [128, 512]) for i in range(2)]
    gxt = [sb(f"gxt{i}", [64, 512], DTX) for i in range(2)]
    gates = [sb(f"gates{i}", [64, 512]) for i in range(2)]
    tanhc = [sb(f"tanhc{i}", [64, 128]) for i in range(2)]
    hsb = [sb(f"hsb{i}", [64, 128]) for i in range(2)]
    hob = [sb(f"hob{i}", [64, 128], DTX) for i in range(2)] if bf16 else hsb
    t1 = sb("t1", [64, 128])
    t2 = sb("t2", [64, 128])
    cst = sb("cst", [64, 128])
    idbuf = sb("idbuf", [128, 64])
    ident_f32 = sb("ident_f32", [64, 64])

    p1b = [ps_(f"p1b{m}", [128, 512]) for m in range(8)]
    psg = [p1b[0], p1b[1]]                                     # rec gates [64,512]
    psh = [p1b[2], p1b[3]]                                     # hT transpose [128, :64]

    dma_w = sem("dma_w")
    dma_x = [sem(f"dma_x{i}") for i in range(3)]
    dma_gxA = [sem("dma_gxA0"), sem("dma_gxA1")]
    cc_sem = sem("cc_sem")
    dma_gx = [sem("dma_gx0"), sem("dma_gx1")]
    dma_out = sem("dma_out")
    hob_s = sem("hob_s")
    dma_id = sem("dma_id")
    pe_p1 = sem("pe_p1")
    act_p1 = sem("act_p1")
    pe_s = sem("pe_s")
    dve_s = sem("dve_s")
    act_s = sem("act_s")
    prep_s = sem("prep_s")
    rsem_all = sem("rsem_all")
    rsem = [sem(f"rsem{m}") for m in range(1, 8)] if safe_rsem else None
    lsem = [sem(f"lsem{m}") for m in range(1, 8)]
    id_rsem = [sem(f"idr{m}") for m in range(1, 8)]
    id_lsem = [sem(f"idl{m}") for m in range(1, 8)]

    INIT_DMAS = 9 * 16  # wih whh bias ones ident identf hbuf0 c rank
    LSEM_CHK = 16       # check send-drain lag every LSEM_CHK steps

    with nc.Block() as block:

        # ---------------- SYNC ----------------
        @block.sync
        def _(sync):
            sync.dma_start(out=wih_sb[:, :], in_=wih[:, :]).then_inc(dma_w, 16)
            sync.dma_start(out=whh_sb[:, :], in_=whh[:, :]).then_inc(dma_w, 16)
            sync.dma_start(out=bias_sb[:, :], in_=biasd[:, :]).then_inc(dma_w, 16)
            sync.dma_start(out=ones_sb[:, :], in_=onesd[:, :]).then_inc(dma_w, 16)
            sync.dma_start(out=ident_sb[:, :], in_=identd[:, :]).then_inc(dma_w, 16)
            sync.dma_start(out=ident_f32[:, :], in_=identfd[:, :]).then_inc(dma_w, 16)
            sync.dma_start(out=hbuf[0][:, :], in_=zerod[:, :]).then_inc(dma_w, 16)
            sync.dma_start(out=cst[:, :], in_=zerod[:64, 0:128]).then_inc(dma_w, 16)
            sync.dma_start(out=idbuf[:, 0:8], in_=rankd[:, :]).then_inc(dma_w, 16)
            # phase 1 x tiles (local shard)
            for T in range(min(3, TT)):
                sync.dma_start(
                    out=bass.AP(xtile[T % 3], 0, [[1024, 128], [128, 8], [1, 128]]),
                    in_=bass.AP(xT, T * 128, [[TOK_L, 128], [128 * TOK_L, 8], [1, 128]]),
                ).then_inc(dma_x[T % 3], 16)
            for T in range(TT):
                if T + 3 < TT:
                    sync.wait_ge(pe_p1, 8 * (T + 1))
                    sync.dma_start(
                        out=bass.AP(xtile[(T + 3) % 3], 0, [[1024, 128], [128, 8], [1, 128]]),
                        in_=bass.AP(xT, (T + 3) * 128, [[TOK_L, 128], [128 * TOK_L, 8], [1, 128]]),
                    ).then_inc(dma_x[(T + 3) % 3], 16)
                # one strided DMA: stag (8 gate-chunks of this tile) -> gxA
                # chunk j rows: j*TOK_L + T*128 .. +128
                sync.wait_ge(act_p1, 8 * (T + 1))
                sync.dma_start(
                    out=bass.AP(gxA, T * 128 * 512, [[512, 128], [TOK_L * 512, 8], [1, 512]]),
                    in_=bass.AP(stag[T % 2], 0, [[4096, 128], [512, 8], [1, 512]]),
                ).then_inc(dma_gxA[T % 2], 16)
            # phase 2 gxt prefetch: gated on the AllToAll. The out-DMA (shifted
            # 2 steps back) also lives here so it stays off the ACT engine.
            sync.wait_ge(cc_sem, 1)
            if ph2:
                for s in range(S):
                    if s >= 2:
                        sync.wait_ge(pe_s, 2 * (s - 2) + 1)
                    sync.dma_start(out=gxt[s % 2][:, :], in_=gxB[s * 64:(s + 1) * 64, :]
                                   ).then_inc(dma_gx[s % 2], 16)
                    if bf16 and s >= 2:
                        sync.wait_ge(hob_s, s - 1)
                        sync.dma_start(out=out[s - 2, :, :], in_=hob[(s - 2) % 2][:, :]
                                       ).then_inc(dma_out, 16)
                if bf16:
                    for s in (S - 2, S - 1):
                        sync.wait_ge(hob_s, s + 1)
                        sync.dma_start(out=out[s, :, :], in_=hob[s % 2][:, :]
                                       ).then_inc(dma_out, 16)

        # ---------------- PE ----------------
        @block.tensor
        def _(tensor):
            tensor.wait_ge(dma_w, INIT_DMAS)
            # phase 1: bank g = my tokens x gate-chunk g (all cores' gates)
            for T in range(TT):
                tensor.wait_ge(dma_x[T % 3], 16 * (T // 3 + 1))
                for g in range(8):
                    if T >= 1:
                        tensor.wait_ge(act_p1, 8 * (T - 1) + g + 1)
                    for j in range(8):
                        tensor.matmul(
                            p1b[g][:, :],
                            xtile[T % 3][:, j * 128:(j + 1) * 128],
                            wih_sb[:, (j * 8 + g) * 512:(j * 8 + g + 1) * 512],
                            start=(j == 0), stop=False,
                        )
                    mm = tensor.matmul(p1b[g][:, :], ones_sb[:, :],
                                       bias_sb[:, g * 512:(g + 1) * 512],
                                       start=False, stop=True)
                    mm.then_inc(pe_p1, 1)
            # phase 2
            tensor.wait_ge(act_p1, 8 * TT)
            for s in range(S if ph2 else 0):
                par = s % 2
                # gates psum [64, 512]
                tensor.wait_ge(dma_gx[par], 16 * (s // 2 + 1))
                if s >= 2:
                    tensor.wait_ge(act_s, 3 * (s - 2) + 2)   # psum WAR (acts of s-2 done)
                tensor.matmul(psg[par][0:64, :], ident_sb[:, :], gxt[par][:, :],
                              start=True, stop=False)
                if s >= 1:
                    tensor.wait_ge(dve_s, 5 * s)             # own hT slot ready
                    if rsem_wait:
                        if safe_rsem:
                            for m in range(7):
                                tensor.wait_ge(rsem[m], 2 * s)
                        else:
                            tensor.wait_ge(rsem_all, 14 * s)
                for j in range(8):
                    mm = tensor.matmul(
                        psg[par][0:64, :],
                        hbuf[par][:, j * 64:(j + 1) * 64],
                        whh_sb[:, j * 512:(j + 1) * 512],
                        start=False, stop=(j == 7),
                    )
                    if j == 7:
                        mm.then_inc(pe_s, 1)
                # wait h of this step then transpose to hT slice
                tensor.wait_ge(dve_s, 5 * s + 4)
                tensor.transpose(psh[(s + 1) % 2][:, 0:64], hsb[par][:, :], ident_f32[:, :]
                                 ).then_inc(pe_s, 1)
            if ph2:
                tensor.wait_ge(act_s, 3 * S)

        # ---------------- ACT ----------------
        @block.scalar
        def _(scalar):
            scalar.wait_ge(dma_w, INIT_DMAS)
            # phase 1 copies psum bank g -> staging chunk g (casts to DTX)
            for T in range(TT):
                if T >= 2:
                    scalar.wait_ge(dma_gxA[T % 2], 16 * (T // 2))
                for g in range(8):
                    scalar.wait_ge(pe_p1, 8 * T + g + 1)
                    scalar.activation(stag[T % 2][:, g * 512:(g + 1) * 512],
                                      p1b[g][:, :], AF.Identity).then_inc(act_p1, 1)
            # idmap
            for m in range(7):
                scalar.wait_ge(id_rsem[m], 2)
            scalar.dma_start(out=idmap[:, :], in_=idbuf[:, :]).then_inc(dma_id, 16)
            # phase 2: sigmoid [64, 384] (i,f,o), tanh g, tanh c; out DMA
            for s in range(S if ph2 else 0):
                par = s % 2
                scalar.wait_ge(pe_s, 2 * s + 1)
                scalar.activation(gates[par][:, 0:384], psg[par][0:64, 0:384], AF.Sigmoid
                                  ).then_inc(act_s, 1)
                scalar.activation(gates[par][:, 384:512], psg[par][0:64, 384:512], AF.Tanh
                                  ).then_inc(act_s, 1)
                scalar.wait_ge(dve_s, 5 * s + 3)
                scalar.activation(tanhc[par][:, :], cst[:, :], AF.Tanh).then_inc(act_s, 1)
                scalar.wait_ge(dve_s, 5 * s + 4)
                if bf16:
                    if s >= 2:
                        scalar.wait_ge(dma_out, 16 * (s - 1))
                    scalar.activation(hob[par][:, :], hsb[par][:, :], AF.Copy
                                      ).then_inc(hob_s, 1)
                else:
                    scalar.dma_start(out=out[s, :, :], in_=hob[par][:, :]
                                     ).then_inc(dma_out, 16)
            if ph2:
                scalar.wait_ge(dma_out, 16 * S)

        # ---------------- DVE ----------------
        @block.vector
        def _(vector):
            vector.wait_ge(dma_w, INIT_DMAS)
            for s in range(S if ph2 else 0):
                par = s % 2
                vector.wait_ge(act_s, 3 * s + 1)
                if s >= 1:
                    vector.wait_ge(dve_s, 5 * (s - 1) + 3)   # prev c written
                vector.tensor_mul(t1[:, :], gates[par][:, 128:256], cst[:, :]).then_inc(dve_s, 1)
                vector.wait_ge(act_s, 3 * s + 2)
                vector.tensor_mul(t2[:, :], gates[par][:, 0:128], gates[par][:, 384:512]
                                  ).then_inc(dve_s, 1)
                vector.wait_ge(dve_s, 5 * s + 2)
                vector.tensor_add(cst[:, :], t1[:, :], t2[:, :]).then_inc(dve_s, 1)
                vector.wait_ge(act_s, 3 * s + 3)
                if s >= 2:
                    if bf16:
                        vector.wait_ge(hob_s, s - 1)   # hob copy of s-2 read hsb
                    else:
                        vector.wait_ge(dma_out, 16 * (s - 1))
                vector.tensor_mul(hsb[par][:, :], gates[par][:, 256:384], tanhc[par][:, :]
                                  ).then_inc(dve_s, 1)
                # copy hT from transpose psum into hbuf slot 0 of next parity
                vector.wait_ge(pe_s, 2 * (s + 1))
                vector.tensor_copy(hbuf[(s + 1) % 2][:, 0:64], psh[(s + 1) % 2][:, 0:64]
                                   ).then_inc(dve_s, 1)

        # ---------------- GPSIMD ----------------
        @block.gpsimd
        def _(gp):
            gp.load_library(library_config.remote_dma)

            NB = 7 if mode == "full" else nbcast
            QN = lambda m: (m - 1) % nq
            QTRIG = [sum(1 for m in range(1, NB + 1) if QN(m) == q)
                     for q in range(nq)]

            def bcast(m, out_ap, in_ap, rs, ls):
                rdests = [None] * 8
                rdests[m] = (0, m)
                if sem_only:
                    gp.remote_sem_update_broadcast(
                        remote_sem=rs, local_sem=ls, rdests=rdests,
                        queue_num=QN(m)).then_inc(prep_s, 1)
                else:
                    gp.remote_dma_broadcast(out_ap=out_ap, in_ap=in_ap,
                                            remote_sem=rs, local_sem=ls, rdests=rdests,
                                            queue_num=QN(m)).then_inc(prep_s, 1)

            def fire():
                for q in range(nq):
                    if QTRIG[q]:
                        gp.trigger_dma(count=QTRIG[q], queue_num=q)

            for m in range(1, 8):
                rdests = [None] * 8
                rdests[m] = (0, m)
                gp.remote_dma_broadcast(out_ap=idbuf[:, m * 8:(m + 1) * 8],
                                        in_ap=idbuf[:, 0:8],
                                        remote_sem=id_rsem[m - 1],
                                        local_sem=id_lsem[m - 1], rdests=rdests,
                                        ).then_inc(prep_s, 1)
            gp.wait_ge(prep_s, 7)
            gp.wait_ge(dma_w, INIT_DMAS)
            gp.trigger_dma(count=7)
            # redistribute gx: [my tokens, all gates] -> [all tokens, my gates]
            gp.wait_ge(dma_gxA[0], 16 * ((TT + 1) // 2))
            if TT >= 2:
                gp.wait_ge(dma_gxA[1], 16 * (TT // 2))
            gp.collective_compute(
                "AllToAll",
                mybir.AluOpType.bypass,
                replica_groups=[list(range(NC))],
                ins=[gxA.ap().opt()],
                outs=[gxB.ap().opt()],
            ).then_inc(cc_sem, 1)
            gp.wait_ge(cc_sem, 1)
            if do_exchange and ph2 and NB > 0:
                if S >= 2:
                    for m in range(1, NB + 1):
                        bcast(m, hbuf[1][:, m * 64:(m + 1) * 64], hbuf[1][:, 0:64],
                              rsem[m - 1] if safe_rsem else rsem_all, lsem[m - 1])
                for s in range(S):
                    gp.wait_ge(dve_s, 5 * (s + 1))
                    if s < S - 1:
                        gp.wait_ge(prep_s, 7 + NB * (s + 1))
                        fire()
                    if s % LSEM_CHK == 0 and s >= LSEM_CHK:
                        for m in range(NB):
                            gp.wait_ge(lsem[m], 16 * (s - LSEM_CHK + 1))
                    if s + 1 <= S - 2:
                        p2 = (s + 2) % 2
                        for m in range(1, NB + 1):
                            bcast(m, hbuf[p2][:, m * 64:(m + 1) * 64], hbuf[p2][:, 0:64],
                                  rsem[m - 1] if safe_rsem else rsem_all, lsem[m - 1])
                for m in range(NB):
                    gp.wait_ge(lsem[m], 16 * (S - 1))

    es.close()
    library_overlay.lower_extended_insts(nc)
    return nc


def host_prepare(x, W_ih, W_hh, b_ih, b_hh, S, umap, bf16=True):
    TOK = S * B
    TOK_L = TOK // NC
    import ml_dtypes
    dtx = ml_dtypes.bfloat16 if bf16 else np.float32
    xs = np.ascontiguousarray(x[:, :S, :])
    xT = np.ascontiguousarray(xs.transpose(2, 1, 0).reshape(I, TOK)).astype(dtx)
    # gate col order (i, f, o, g): source gate blocks in W_* rows: i=0,f=1,g=2,o=3
    gorder = [0, 1, 3, 2]
    rows_of = [np.concatenate(
        [np.arange(g * H + c * HS, g * H + (c + 1) * HS) for g in gorder])
        for c in range(NC)]
    # rank-ordered full-width W_ihT / bias: chunk c = gates of core c (same
    # for every core; the AllToAll routes by rank)
    wih_full = np.concatenate([W_ih[rows_of[c], :].T for c in range(NC)], axis=1)
    wih_packed = np.ascontiguousarray(
        wih_full.reshape(8, 128, 8, 512).transpose(1, 0, 2, 3).reshape(128, 32768)
    ).astype(dtx)
    bias_full = (b_ih + b_hh).astype(np.float32)
    bias_perm = np.concatenate([bias_full[rows_of[c]] for c in range(NC)]).reshape(1, 4096)
    in_maps = []
    for r in range(NC):
        rows = rows_of[r]
        whh_slice = W_hh[rows, :]                            # [512 g, 1024 k]
        whh_perm = np.concatenate(
            [whh_slice[:, umap[r][j] * HS:(umap[r][j] + 1) * HS].T for j in range(8)],
            axis=0)                                          # [1024 (slot-k), 512]
        in_maps.append(dict(
            xT=np.ascontiguousarray(xT[:, r * TOK_L:(r + 1) * TOK_L]),
            wih=wih_packed,
            whh=np.ascontiguousarray(whh_perm.reshape(8, 128, 512).transpose(1, 0, 2)
                                     .reshape(128, 4096)).astype(np.float32),
            bias=bias_perm,
            ones=np.ones((1, 128), np.float32),
            ident=np.eye(64).astype(dtx),
            identf=np.eye(64, dtype=np.float32),
            zeros=np.zeros((128, 512), np.float32),
            rankvec=np.full((128, 8), float(r), np.float32),
        ))
    return in_maps


def host_gather(results, S):
    outs = np.stack([np.asarray(results[r]["out"]).astype(np.float32).reshape(S, 64, 128)
                     for r in range(NC)])
    return np.ascontiguousarray(outs.transpose(2, 1, 0, 3).reshape(B, S, H))


def read_umap(results):
    return [[int(round(float(np.asarray(results[r]["idmap"]).reshape(128, 64)[0, m * 8])))
             for m in range(8)] for r in range(NC)]


DEFAULT_UMAP = [[r ^ (m if m < 4 else m ^ 2) for m in range(8)] for r in range(NC)]
IDENTITY_UMAP = [[r ^ m for m in range(8)] for r in range(NC)]



# ---------------- PJRT runner (no donation, outputs as results) ----------------

import jax
from jax.sharding import Mesh, PartitionSpec
from jax.experimental.shard_map import shard_map
from concourse import bass2jax

import jax
from jax.sharding import Mesh, PartitionSpec
from jax.experimental.shard_map import shard_map
from concourse import bass2jax


class Runner:
    def __init__(self, nc, n_cores=8, pass_out_zeros=False):
        # pass_out_zeros: ship zero buffers for ExternalOutputs (needed only
        # if the kernel does not write every output element)
        bass2jax.install_neuronx_cc_hook()
        self.nc = nc
        self.n_cores = n_cores
        partition_name = nc.partition_id_tensor.name if nc.partition_id_tensor else None
        in_names, out_names, out_avals = [], [], []
        for alloc in nc.m.functions[0].allocations:
            if not isinstance(alloc, mybir.MemoryLocationSet):
                continue
            name = alloc.memorylocations[0].name
            if alloc.kind == "ExternalInput":
                if name != partition_name:
                    in_names.append(name)
            elif alloc.kind == "ExternalOutput":
                out_names.append(name)
                out_avals.append(jax.core.ShapedArray(
                    tuple(alloc.tensor_shape), mybir.dt.np(alloc.dtype)))
        self.in_names, self.out_names, self.out_avals = in_names, out_names, out_avals
        self.pass_out_zeros = pass_out_zeros
        n_params = len(in_names)
        n_outs = len(out_names) if pass_out_zeros else 0
        in_names_all = (in_names + (out_names if pass_out_zeros else [])
                        + ([partition_name] if partition_name else []))

        def _body(*args):
            operands = list(args)
            if partition_name is not None:
                operands.append(bass2jax.partition_id_tensor())
            outs = bass2jax._bass_exec_p.bind(
                *operands, out_avals=tuple(out_avals), in_names=tuple(in_names_all),
                out_names=tuple(out_names), lowering_input_output_aliases=(),
                sim_require_finite=False, sim_require_nnan=False, nc=nc)
            return tuple(outs)

        devices = jax.devices()[:n_cores]
        self.mesh = Mesh(np.asarray(devices), ("core",))
        self.jitted = jax.jit(shard_map(
            _body, mesh=self.mesh,
            in_specs=(PartitionSpec("core"),) * (n_params + n_outs),
            out_specs=(PartitionSpec("core"),) * len(out_names), check_rep=False),
            keep_unused=True)
        self.dev_args = None

    def set_inputs(self, in_maps, only=None):
        n = self.n_cores
        if self.dev_args is None:
            n_extra = len(self.out_names) if self.pass_out_zeros else 0
            self.dev_args = [None] * (len(self.in_names) + n_extra)
            only = None
        for i, name in enumerate(self.in_names):
            if only is not None and name not in only:
                continue
            cat = np.concatenate([np.asarray(in_maps[c][name]) for c in range(n)], axis=0)
            self.dev_args[i] = jax.device_put(cat)
        if only is None and self.pass_out_zeros:
            for j, av in enumerate(self.out_avals):
                z = np.zeros((n * av.shape[0], *av.shape[1:]), av.dtype)
                self.dev_args[len(self.in_names) + j] = jax.device_put(z)

    def run(self):
        outs = self.jitted(*self.dev_args)
        jax.block_until_ready(outs)
        return outs

    def results(self, outs):
        n = self.n_cores
        res = []
        for c in range(n):
            d = {}
            for i, name in enumerate(self.out_names):
                a = np.asarray(outs[i])
                d[name] = a.reshape(n, *self.out_avals[i].shape)[c]
            res.append(d)
        return res



# ---------------- harness entry point ----------------


USE_BF16 = True      # bf16 x / W_ih / gx / output staging; recurrence fp32
_CACHE = {}


def kernel(x, W_ih, W_hh, b_ih, b_hh):
    """Full-input distributed LSTM on 8 trn2 NeuronCores. Returns (B, S, H) f32."""
    x = np.ascontiguousarray(np.asarray(x, np.float32))
    W_ih = np.asarray(W_ih, np.float32)
    W_hh = np.asarray(W_hh, np.float32)
    b_ih = np.asarray(b_ih, np.float32)
    b_hh = np.asarray(b_hh, np.float32)
    S = x.shape[1]

    if "runner" not in _CACHE:
        nc = build_nc(S, bf16=USE_BF16)
        _CACHE["runner"] = Runner(nc, NC)
        _CACHE["S"] = S
    assert _CACHE["S"] == S
    r = _CACHE["runner"]

    umap = _CACHE.get("umap", DEFAULT_UMAP)
    in_maps = host_prepare(x, W_ih, W_hh, b_ih, b_hh, S, umap, bf16=USE_BF16)
    r.set_inputs(in_maps)
    res = r.results(r.run())
    obs = read_umap(res)
    if obs != umap:
        # physical core mapping differs from assumption: rebuild the permuted
        # recurrent weights and re-execute (same compiled NEFF)
        assert all(sorted(row) == list(range(8)) for row in obs), obs
        _CACHE["umap"] = obs
        in_maps = host_prepare(x, W_ih, W_hh, b_ih, b_hh, S, obs, bf16=USE_BF16)
        r.set_inputs(in_maps, only={"whh"})
        res = r.results(r.run())
        assert read_umap(res) == obs
    return host_gather(res, S)

